# revision 26
# baseline (speedup 1.0000x reference)
"""DigitCaps dynamic-routing kernel for 8 Trainium2 NeuronCores.

Strategy: shard num_route_nodes (R=2048 -> 256/core), fp16 throughout.
  - Phase 0: s1 = sum_r u_r as one K=128-packed matmul accumulation chain
    over all local routes (u never materialized). AllReduce -> v1.
  - Phase 1: per 8-route tile: produce u on the PE (fp16 inputs), drain
    PSUM->SBUF on the scalar engine, write the u tile to DRAM, and run the
    iteration-2 routing on the tile while it is still in SBUF.
  - Phase 2 (per extra iteration): stream u tiles back once, same routing.

Routing per tile -- both contractions ride the tensor engine:
  p = u*v                              (DVE, fp16 2x mode)
  dot: identity-matmul transposes of p 128-col chunks PSUM-accumulate the
       m-subgroups per route (dotp[(m%4)*32+c, b]), then one matmul against
       a block-ones matrix finishes sum_m and lands dot[b, (r,c)] directly.
  softmax over caps                    (DVE + ACT exp, per-(b,r) max)
  q = u*coef                           (split DVE/Pool)
  s: identity-matmul transposes of q PSUM-accumulate over routes AND tiles
     into s_T[cm%128, (chunk, b)] -- the whole r-contraction costs zero
     vector cycles.
s_T is AllReduced (fp16) across cores; squash runs in the transposed layout
and v returns to [b, cm] via more identity matmuls.  PE emission is
software-pipelined (1-2 tile skew) so no engine head-of-line blocks another.
"""

import os
import sys

if "/opt/trn_rl_repo" not in sys.path:
    sys.path.insert(0, "/opt/trn_rl_repo")

import numpy as np

B, R, K, C, M = 128, 2048, 64, 32, 32
CM = C * M
N_CORES = 8
R_LOC = R // N_CORES
RT = 8                                         # routes per tile
PAIRS = RT // 2
NT = R_LOC // RT
NCH = CM // 128                                # 128-col chunks per route (8)
POOL_Q = int(os.environ.get("DC_POOLQ", "4"))  # of 8 tiles: q-mult on Pool
AR16 = os.environ.get("DC_AR16", "0") == "1"   # fp16 collectives
SIM_MODE = os.environ.get("DC_SIM", "0") == "1"

_compiled = {}
LAST_RESULT = None


def _view(ap, dims):
    """Free-dim view of an AP: keep its partition dim, replace free dims by
    [step, count] pairs (element steps). step 0 = broadcast."""
    import concourse.bass as bass

    return bass.AP(
        tensor=ap.tensor,
        offset=ap.offset,
        ap=[list(ap.ap[0])] + [[s, c] for s, c in dims],
    )


def _ap(ap, dims):
    """Fully custom AP (all dims given) at the base offset of `ap`."""
    import concourse.bass as bass

    return bass.AP(
        tensor=ap.tensor,
        offset=ap.offset,
        ap=[[s, c] for s, c in dims],
    )


def _build(n_iters, repeat=1):
    import concourse.mybir as mybir
    import concourse.tile as tile
    from concourse import bacc

    f32 = mybir.dt.float32
    f16 = mybir.dt.float16
    ar_dt = f16 if AR16 else f32
    op = mybir.AluOpType
    AX = mybir.AxisListType
    ACT = mybir.ActivationFunctionType

    nc = bacc.Bacc("TRN2", target_bir_lowering=False, debug=False,
                   num_devices=1 if SIM_MODE else N_CORES)
    xT = nc.dram_tensor("xT", [R_LOC // 2, 2, K, B], f16,
                        kind="ExternalInput").ap()
    wT = nc.dram_tensor("wT", [R_LOC // 2, 2, K, CM], f16,
                        kind="ExternalInput").ap()
    idin = nc.dram_tensor("idin", [128, 128], f16, kind="ExternalInput").ap()
    idin32 = nc.dram_tensor("idin32", [128, 128], f32,
                            kind="ExternalInput").ap()
    obin = nc.dram_tensor("obin", [128, C], f32, kind="ExternalInput").ap()
    out = nc.dram_tensor("out", [B, CM], f32, kind="ExternalOutput").ap()
    DEBUG = os.environ.get("DC_DEBUG", "0") == "1"
    if DEBUG:
        dbg = nc.dram_tensor("dbg", [B, R_LOC * C], mybir.dt.float16,
                             kind="ExternalOutput").ap()

    with tile.TileContext(nc) as tc:
        with (
            tc.tile_pool(name="sm", bufs=2) as sm,
            tc.tile_pool(name="persist", bufs=1) as persist,
            tc.tile_pool(name="dram", bufs=1, space="DRAM") as dram,
            tc.tile_pool(name="drbounce", bufs=min(2 * (n_iters + 1) * repeat, 8),
                         space="DRAM") as drb,
        ):
            u_dram = dram.tile([B, R_LOC * CM], f16)
            b_log = persist.tile([B, R_LOC * C], f16)   # logits, layout (r, c)
            v_sb = persist.tile([B, CM], f32)
            v_u = persist.tile([B, CM], f16)
            ident = persist.tile([128, 128], f16)
            ident32 = persist.tile([128, 128], f32)
            oblk = persist.tile([128, C], f32)          # [p,c]=1 iff p%32==c
            nc.sync.dma_start(ident[:], idin)
            nc.sync.dma_start(ident32[:], idin32)
            nc.sync.dma_start(oblk[:], obin)

            def squash_T(s_sbT, scale):
                """squash on s_T[cm%128, (chunk, b)]; writes v_u/v_sb[b, cm].

                norm over m: chunk ci holds m in [4ci,4ci+4), partition
                p = (m%4)*32 + c."""
                with tc.tile_pool(name="vps", bufs=1, space="PSUM") as vps:
                    if scale != 1.0:
                        nc.vector.tensor_scalar(s_sbT[:], s_sbT[:], scale,
                                                None, op0=op.mult)
                    sq2 = sm.tile([128, NCH * B], f32, tag="sq2")
                    nc.vector.tensor_tensor(sq2[:], s_sbT[:], s_sbT[:],
                                            op=op.mult)
                    sqp = sm.tile([128, B], f32, tag="sqp")
                    nc.vector.tensor_reduce(
                        sqp[:], _view(sq2[:], [(1, B), (B, NCH)]), axis=AX.X,
                        op=op.add)
                    spt = vps.tile([128, 128], f32, tag="spt")
                    nc.tensor.matmul(spt[:], sqp[:], ident32[:],
                                     start=True, stop=True)
                    sq = sm.tile([B, C], f32, tag="sq")
                    nc.vector.tensor_reduce(
                        sq[:], _view(spt[:], [(1, C), (C, 4)]), axis=AX.X,
                        op=op.add)
                    rt = sm.tile([B, C], f32, tag="rt")
                    nc.scalar.activation(rt[:], sq[:], ACT.Sqrt)
                    nc.vector.tensor_scalar(rt[:], rt[:], 1e-8, None,
                                            op0=op.add)
                    den = sm.tile([B, C], f32, tag="den")
                    nc.vector.tensor_scalar(den[:], sq[:], 1.0, None,
                                            op0=op.add)
                    nc.vector.tensor_tensor(den[:], den[:], rt[:], op=op.mult)
                    fi = sm.tile([B, C], f32, tag="fi")
                    nc.vector.reciprocal(fi[:], den[:])
                    nc.vector.tensor_tensor(fi[:], fi[:], sq[:], op=op.mult)
                    # transpose s back to [b, cm] and apply fi
                    if s_sbT.dtype != f16:
                        s16 = sm.tile([128, NCH * B], f16, tag="s16")
                        nc.vector.tensor_copy(s16[:], s_sbT[:])
                        s16v = s16
                    else:
                        s16v = s_sbT
                    vp = vps.tile([B, CM], f32, tag="vp")
                    for ci in range(NCH):
                        nc.tensor.matmul(
                            vp[:, ci * 128:(ci + 1) * 128],
                            s16v[:, ci * B:(ci + 1) * B], ident[:],
                            start=(ci % 4 == 0), stop=(ci % 4 == 3))
                    fi_b = _view(fi[:], [(0, M), (1, C)])
                    vp_v = _view(vp[:], [(C, M), (1, C)])
                    nc.vector.tensor_tensor(
                        _view(v_u[:], [(C, M), (1, C)]), vp_v, fi_b,
                        op=op.mult)
                    nc.vector.tensor_tensor(
                        _view(v_sb[:], [(C, M), (1, C)]), vp_v, fi_b,
                        op=op.mult)

            def allreduce_squash_T(sT_psum, scale):
                sT_sb = sm.tile([128, NCH * B], ar_dt, tag="sT_sb")
                nc.vector.tensor_copy(sT_sb[:], sT_psum[:])
                bin_ = drb.tile([128, NCH * B], ar_dt, tag="bin")
                bout = drb.tile([128, NCH * B], ar_dt, tag="bout")
                nc.sync.dma_start(bin_[:], sT_sb[:])
                if SIM_MODE:
                    nc.sync.dma_start(bout[:], bin_[:])
                else:
                    nc.gpsimd.collective_compute(
                        "AllReduce", op.add,
                        replica_groups=[list(range(N_CORES))],
                        ins=[bin_.opt()], outs=[bout.opt()],
                    )
                s2 = sm.tile([128, NCH * B], ar_dt, tag="s2")
                nc.sync.dma_start(s2[:], bout[:])
                squash_T(s2, scale)

            def emit_phase0(sT_pool):
                """s1 = sum_r u_r via one PSUM chain; transpose into s_T."""
                with (
                    tc.tile_pool(name="x0", bufs=3) as x0,
                    tc.tile_pool(name="w0", bufs=3) as w0,
                    tc.tile_pool(name="s1p", bufs=1, space="PSUM") as s1p,
                ):
                    s1_psum = s1p.tile([B, CM], f32)
                    for t in range(NT):
                        xt = x0.tile([2 * K, PAIRS * B], f16)
                        nc.sync.dma_start(
                            xt[:],
                            _ap(xT[t * PAIRS:(t + 1) * PAIRS],
                                [(B, 2 * K), (2 * K * B, PAIRS), (1, B)]))
                        wt = w0.tile([2 * K, PAIRS * CM], f16)
                        nc.sync.dma_start(
                            wt[:],
                            _ap(wT[t * PAIRS:(t + 1) * PAIRS],
                                [(CM, 2 * K), (2 * K * CM, PAIRS), (1, CM)]))
                        for j in range(PAIRS):
                            first = (t == 0 and j == 0)
                            last = (t == NT - 1 and j == PAIRS - 1)
                            for h in range(2):
                                nc.tensor.matmul(
                                    s1_psum[:, h * 512:(h + 1) * 512],
                                    xt[:, j * B:(j + 1) * B],
                                    wt[:, j * CM + h * 512:j * CM + (h + 1) * 512],
                                    start=first, stop=last,
                                )
                    s1_16 = sm.tile([B, CM], f16, tag="s1_16")
                    nc.scalar.copy(s1_16[:], s1_psum[:])
                    sT = sT_pool.tile([128, NCH * B], f32, tag="sT")
                    for ci in range(NCH):
                        nc.tensor.matmul(
                            sT[:, ci * B:(ci + 1) * B],
                            s1_16[:, ci * 128:(ci + 1) * 128], ident[:],
                            start=(ci % 4 == 0), stop=(ci % 4 == 3))
                return sT

            def dve_tile(t, ut, it, p_pool, q_pool, dp_pool, db_pool):
                """Vector-side routing for one tile; PE parts emitted
                separately (skewed). Returns (p, dotp, dotp32, dot_b, q)."""
                p = p_pool.tile([B, RT * CM], f16, tag="p")
                nc.vector.tensor_tensor(
                    p[:], ut[:], _view(v_u[:], [(0, RT), (1, CM)]), op=op.mult)
                dotp = dp_pool.tile([128, RT * B], f32)      # PSUM, per-r regions
                dotp32 = sm.tile([128, RT * B], f32, tag="dotp32")
                dot_b = db_pool.tile([B, RT * C], f32)       # PSUM
                return p, dotp, dotp32, dot_b

            def pe_dot(p, dotp, dotp32, dot_b):
                """sum_m on the PE: transpose-accumulate p chunks per route,
                drain via ACT, finish with the block-ones matmul."""
                for g in range(RT // 4):
                    for r in range(g * 4, g * 4 + 4):
                        for ci in range(NCH):
                            nc.tensor.matmul(
                                dotp[:, r * B:(r + 1) * B],
                                p[:, r * CM + ci * 128:r * CM + (ci + 1) * 128],
                                ident[:],
                                start=(r % 4 == 0 and ci == 0),
                                stop=(r % 4 == 3 and ci == NCH - 1))
                    nc.scalar.copy(dotp32[:, g * 4 * B:(g + 1) * 4 * B],
                                   dotp[:, g * 4 * B:(g + 1) * 4 * B])
                for r in range(RT):
                    nc.tensor.matmul(
                        dot_b[:, r * C:(r + 1) * C],
                        dotp32[:, r * B:(r + 1) * B], oblk[:],
                        start=(r == 0), stop=(r == RT - 1))

            def softmax_q(t, ut, it, dot_b, q_pool):
                """softmax over caps from dot_b (PSUM) + q-mult."""
                blt = b_log[:, t * RT * C:(t + 1) * RT * C]
                dot_v = _view(dot_b[:], [(C, RT), (1, C)])
                if it == 2:
                    lg_v = dot_v
                    if n_iters > 2:
                        nc.scalar.copy(_view(blt, [(C, RT), (1, C)]), dot_v)
                else:
                    lg = sm.tile([B, RT * C], f16, tag="lg")
                    lg_v = _view(lg[:], [(C, RT), (1, C)])
                    nc.vector.tensor_tensor(
                        lg_v, _view(blt, [(C, RT), (1, C)]), dot_v, op=op.add)
                    if it < n_iters:
                        nc.scalar.copy(_view(blt, [(C, RT), (1, C)]), lg_v)
                mx = sm.tile([B, RT], f32, tag="mx")
                nc.vector.tensor_reduce(mx[:], lg_v, axis=AX.X, op=op.max)
                e = sm.tile([B, RT * C], f32, tag="e")
                e_v = _view(e[:], [(C, RT), (1, C)])
                nc.vector.tensor_tensor(
                    e_v, lg_v, _view(mx[:], [(1, RT), (0, C)]), op=op.subtract)
                nc.scalar.activation(e[:], e[:], ACT.Exp)
                z = sm.tile([B, RT], f32, tag="z")
                nc.vector.tensor_reduce(z[:], e_v, axis=AX.X, op=op.add)
                nc.vector.reciprocal(z[:], z[:])
                coef = sm.tile([B, RT * C], f16, tag="coef")
                nc.vector.tensor_tensor(
                    _view(coef[:], [(C, RT), (1, C)]), e_v,
                    _view(z[:], [(1, RT), (0, C)]), op=op.mult)
                if DEBUG and it == n_iters:
                    nc.scalar.copy(_view(blt, [(C, RT), (1, C)]),
                                   _view(coef[:], [(C, RT), (1, C)]))
                q = q_pool.tile([B, RT * CM], f16, tag="q")
                q_eng = nc.gpsimd if (t % 8) < POOL_Q else nc.vector
                q_eng.tensor_tensor(
                    _view(q[:], [(CM, RT), (C, M), (1, C)]),
                    _view(ut[:], [(CM, RT), (C, M), (1, C)]),
                    _view(coef[:], [(C, RT), (0, M), (1, C)]),
                    op=op.mult)
                return q

            def pe_rsum(q, t, sT):
                """r-sum on the PE: transpose-accumulate q chunks into s_T."""
                for r in range(RT):
                    for ci in range(NCH):
                        # start/stop once per 2KB PSUM bank: start lazily
                        # marks the WHOLE bank pending-zero, so each bank
                        # must see exactly one start (its first write)
                        nc.tensor.matmul(
                            sT[:, ci * B:(ci + 1) * B],
                            q[:, r * CM + ci * 128:r * CM + (ci + 1) * 128],
                            ident[:],
                            start=(t == 0 and r == 0 and ci % 4 == 0),
                            stop=(t == NT - 1 and r == RT - 1 and ci % 4 == 3))

            def emit_phase1(sT):
                """u production fused with iteration-2 routing, software-
                pipelined: PE order is u-mms(t), dot-path(t-1), q-rsum(t-2)."""
                stage = {}
                with (
                    tc.tile_pool(name="x1", bufs=3) as x1,
                    tc.tile_pool(name="w1", bufs=2) as w1,
                    tc.tile_pool(name="up", bufs=3) as up,
                    tc.tile_pool(name="pp", bufs=3, space="PSUM") as pp,
                    tc.tile_pool(name="dpp", bufs=1, space="PSUM") as dpp,
                    tc.tile_pool(name="dbp", bufs=1, space="PSUM") as dbp,
                    tc.tile_pool(name="ppool", bufs=2) as p_pool,
                    tc.tile_pool(name="qpool", bufs=2) as q_pool,
                ):
                    for t in range(NT + 2):
                        if t < NT:
                            xt = x1.tile([2 * K, PAIRS * B], f16)
                            nc.sync.dma_start(
                                xt[:],
                                _ap(xT[t * PAIRS:(t + 1) * PAIRS],
                                    [(B, 2 * K), (2 * K * B, PAIRS), (1, B)]))
                            wt = w1.tile([2 * K, PAIRS * CM], f16)
                            nc.sync.dma_start(
                                wt[:],
                                _ap(wT[t * PAIRS:(t + 1) * PAIRS],
                                    [(CM, 2 * K), (2 * K * CM, PAIRS),
                                     (1, CM)]))
                            ut = up.tile([B, RT * CM], f16)
                            for r_idx in range(RT):
                                j, par = divmod(r_idx, 2)
                                for h in range(2):
                                    ps = pp.tile([B, 512], f32)
                                    nc.tensor.matmul(
                                        ps[:],
                                        xt[par * K:(par + 1) * K,
                                           j * B:(j + 1) * B],
                                        wt[par * K:(par + 1) * K,
                                           j * CM + h * 512:
                                           j * CM + (h + 1) * 512],
                                        start=True, stop=True,
                                    )
                                    nc.scalar.copy(
                                        ut[:, r_idx * CM + h * 512:
                                           r_idx * CM + (h + 1) * 512], ps[:])
                            nc.sync.dma_start(
                                u_dram[:, t * RT * CM:(t + 1) * RT * CM],
                                ut[:])
                            stage[t] = [ut, None, None]
                        if t - 1 >= 0 and t - 1 < NT:
                            ut1 = stage[t - 1][0]
                            p, dotp, dotp32, dot_b = dve_tile(
                                t - 1, ut1, 2, p_pool, q_pool, dpp, dbp)
                            pe_dot(p, dotp, dotp32, dot_b)
                            q = softmax_q(t - 1, ut1, 2, dot_b, q_pool)
                            stage[t - 1][1] = q
                        if t - 2 >= 0:
                            pe_rsum(stage[t - 2][1], t - 2, sT)
                            del stage[t - 2]

            def emit_phase2(it, sT):
                """One streaming routing pass over staged u (1-tile skew on
                the PE q-rsum)."""
                stage = {}
                with (
                    tc.tile_pool(name="up2", bufs=4) as up,
                    tc.tile_pool(name="dpp2", bufs=1, space="PSUM") as dpp,
                    tc.tile_pool(name="dbp2", bufs=2, space="PSUM") as dbp,
                    tc.tile_pool(name="ppool2", bufs=2) as p_pool,
                    tc.tile_pool(name="qpool2", bufs=2) as q_pool,
                ):
                    for t in range(NT + 1):
                        if t < NT:
                            ut = up.tile([B, RT * CM], f16)
                            nc.sync.dma_start(
                                ut[:],
                                u_dram[:, t * RT * CM:(t + 1) * RT * CM])
                            p, dotp, dotp32, dot_b = dve_tile(
                                t, ut, it, p_pool, q_pool, dpp, dbp)
                            pe_dot(p, dotp, dotp32, dot_b)
                            q = softmax_q(t, ut, it, dot_b, q_pool)
                            stage[t] = q
                        if t - 1 >= 0:
                            pe_rsum(stage[t - 1], t - 1, sT)
                            del stage[t - 1]

            def emit_once():
                with tc.tile_pool(name="sTp0", bufs=1, space="PSUM") as sTp:
                    sT = emit_phase0(sTp)
                    allreduce_squash_T(sT, 1.0 / C)
                if n_iters >= 2:
                    with tc.tile_pool(name="sTp1", bufs=1, space="PSUM") as sTp:
                        sT = sTp.tile([128, NCH * B], f32, tag="sT")
                        emit_phase1(sT)
                        allreduce_squash_T(sT, 1.0)
                for it in range(3, n_iters + 1):
                    with tc.tile_pool(name="sTp2", bufs=1, space="PSUM") as sTp:
                        sT = sTp.tile([128, NCH * B], f32, tag="sT")
                        emit_phase2(it, sT)
                        allreduce_squash_T(sT, 1.0)

            for _ in range(repeat):
                emit_once()

            nc.sync.dma_start(out[:], v_sb[:])
            if DEBUG:
                nc.sync.dma_start(dbg, b_log[:])

    nc.compile()
    return nc


def make_in_maps(x, w):
    """Host-side shard + layout prep: fp16, route pairs packed on 128
    partitions, weight columns (m, c) with c innermost."""
    ident = np.eye(128, dtype=np.float16)
    ident32 = np.eye(128, dtype=np.float32)
    oblk = np.zeros((128, C), dtype=np.float32)
    oblk[np.arange(128), np.arange(128) % C] = 1.0
    in_maps = []
    for c in range(N_CORES):
        sl = slice(c * R_LOC, (c + 1) * R_LOC)
        xT_c = np.ascontiguousarray(
            x[:, sl, :].transpose(1, 2, 0).reshape(R_LOC // 2, 2, K, B)
        ).astype(np.float16)
        wT_c = np.ascontiguousarray(
            w[sl].reshape(R_LOC // 2, 2, C, K, M).transpose(0, 1, 3, 4, 2)
        ).reshape(R_LOC // 2, 2, K, CM).astype(np.float16)
        in_maps.append({"xT": xT_c, "wT": wT_c, "idin": ident,
                        "idin32": ident32, "obin": oblk})
    return in_maps


def kernel(x, route_weights, num_iterations):
    global LAST_RESULT
    from concourse import bass_utils

    n = int(num_iterations)
    assert n >= 1
    x = np.asarray(x, dtype=np.float32)
    w = np.asarray(route_weights, dtype=np.float32)
    assert x.shape == (B, R, K) and w.shape == (R, C, K, M)

    if n not in _compiled:
        _compiled[n] = _build(n)
    nc = _compiled[n]

    in_maps = make_in_maps(x, w)
    res = bass_utils.run_bass_kernel_spmd(
        nc, in_maps, core_ids=list(range(N_CORES)))
    LAST_RESULT = res
    return np.ascontiguousarray(
        res.results[0]["out"].reshape(B, M, C).transpose(0, 2, 1)
    ).astype(np.float32)


# revision 29
# speedup vs baseline: 1.0054x; 1.0054x over previous
"""DigitCaps dynamic-routing kernel for 8 Trainium2 NeuronCores.

Strategy: shard num_route_nodes (R=2048 -> 256/core), fp16 throughout.
  - Phase 0: s1 = sum_r u_r as one K=128-packed matmul accumulation chain
    over all local routes (u never materialized). AllReduce -> v1.
  - Phase 1: per 8-route tile: produce u on the PE (fp16 inputs), drain
    PSUM->SBUF on the scalar engine, write the u tile to DRAM, and run the
    iteration-2 routing on the tile while it is still in SBUF.
  - Phase 2 (per extra iteration): stream u tiles back once, same routing.

Routing per tile -- both contractions ride the tensor engine:
  p = u*v                              (DVE, fp16 2x mode)
  dot: identity-matmul transposes of p 128-col chunks PSUM-accumulate the
       m-subgroups per route (dotp[(m%4)*32+c, b]), then one matmul against
       a block-ones matrix finishes sum_m and lands dot[b, (r,c)] directly.
  softmax over caps                    (DVE + ACT exp, per-(b,r) max)
  q = u*coef                           (split DVE/Pool)
  s: identity-matmul transposes of q PSUM-accumulate over routes AND tiles
     into s_T[cm%128, (chunk, b)] -- the whole r-contraction costs zero
     vector cycles.
s_T is AllReduced (fp16) across cores; squash runs in the transposed layout
and v returns to [b, cm] via more identity matmuls.  PE emission is
software-pipelined (1-2 tile skew) so no engine head-of-line blocks another.
"""

import os
import sys

if "/opt/trn_rl_repo" not in sys.path:
    sys.path.insert(0, "/opt/trn_rl_repo")

import numpy as np

B, R, K, C, M = 128, 2048, 64, 32, 32
CM = C * M
N_CORES = 8
R_LOC = R // N_CORES
RT = 8                                         # routes per tile
PAIRS = RT // 2
NT = R_LOC // RT
NCH = CM // 128                                # 128-col chunks per route (8)
QPOOL_R = int(os.environ.get("DC_QPOOL_R", "3"))  # q routes/tile on Pool
AR16 = os.environ.get("DC_AR16", "0") == "1"   # fp16 collectives
SIM_MODE = os.environ.get("DC_SIM", "0") == "1"

_compiled = {}
LAST_RESULT = None


def _view(ap, dims):
    """Free-dim view of an AP: keep its partition dim, replace free dims by
    [step, count] pairs (element steps). step 0 = broadcast."""
    import concourse.bass as bass

    return bass.AP(
        tensor=ap.tensor,
        offset=ap.offset,
        ap=[list(ap.ap[0])] + [[s, c] for s, c in dims],
    )


def _ap(ap, dims):
    """Fully custom AP (all dims given) at the base offset of `ap`."""
    import concourse.bass as bass

    return bass.AP(
        tensor=ap.tensor,
        offset=ap.offset,
        ap=[[s, c] for s, c in dims],
    )


def _build(n_iters, repeat=1):
    import concourse.mybir as mybir
    import concourse.tile as tile
    from concourse import bacc

    f32 = mybir.dt.float32
    f16 = mybir.dt.float16
    ar_dt = f16 if AR16 else f32
    op = mybir.AluOpType
    AX = mybir.AxisListType
    ACT = mybir.ActivationFunctionType

    nc = bacc.Bacc("TRN2", target_bir_lowering=False, debug=False,
                   num_devices=1 if SIM_MODE else N_CORES)
    xT = nc.dram_tensor("xT", [R_LOC // 2, 2, K, B], f16,
                        kind="ExternalInput").ap()
    wT = nc.dram_tensor("wT", [R_LOC // 2, 2, K, CM], f16,
                        kind="ExternalInput").ap()
    idin = nc.dram_tensor("idin", [128, 128], f16, kind="ExternalInput").ap()
    idin32 = nc.dram_tensor("idin32", [128, 128], f32,
                            kind="ExternalInput").ap()
    obin = nc.dram_tensor("obin", [128, C], f32, kind="ExternalInput").ap()
    out = nc.dram_tensor("out", [B, CM], f32, kind="ExternalOutput").ap()
    DEBUG = os.environ.get("DC_DEBUG", "0") == "1"
    if DEBUG:
        dbg = nc.dram_tensor("dbg", [B, R_LOC * C], mybir.dt.float16,
                             kind="ExternalOutput").ap()

    with tile.TileContext(nc) as tc:
        with (
            tc.tile_pool(name="sm", bufs=2) as sm,
            tc.tile_pool(name="persist", bufs=1) as persist,
            tc.tile_pool(name="dram", bufs=1, space="DRAM") as dram,
            tc.tile_pool(name="drbounce", bufs=min(2 * (n_iters + 1) * repeat, 8),
                         space="DRAM") as drb,
        ):
            u_dram = dram.tile([B, R_LOC * CM], f16)
            b_log = persist.tile([B, R_LOC * C], f16)   # logits, layout (r, c)
            v_sb = persist.tile([B, CM], f32)
            v_u = persist.tile([B, CM], f16)
            ident = persist.tile([128, 128], f16)
            ident32 = persist.tile([128, 128], f32)
            oblk = persist.tile([128, C], f32)          # [p,c]=1 iff p%32==c
            nc.sync.dma_start(ident[:], idin)
            nc.sync.dma_start(ident32[:], idin32)
            nc.sync.dma_start(oblk[:], obin)

            def squash_T(s_sbT, scale):
                """squash on s_T[cm%128, (chunk, b)]; writes v_u/v_sb[b, cm].

                norm over m: chunk ci holds m in [4ci,4ci+4), partition
                p = (m%4)*32 + c."""
                with tc.tile_pool(name="vps", bufs=1, space="PSUM") as vps:
                    if scale != 1.0:
                        nc.vector.tensor_scalar(s_sbT[:], s_sbT[:], scale,
                                                None, op0=op.mult)
                    sq2 = sm.tile([128, NCH * B], f32, tag="sq2")
                    nc.vector.tensor_tensor(sq2[:], s_sbT[:], s_sbT[:],
                                            op=op.mult)
                    sqp = sm.tile([128, B], f32, tag="sqp")
                    nc.vector.tensor_reduce(
                        sqp[:], _view(sq2[:], [(1, B), (B, NCH)]), axis=AX.X,
                        op=op.add)
                    spt = vps.tile([128, 128], f32, tag="spt")
                    nc.tensor.matmul(spt[:], sqp[:], ident32[:],
                                     start=True, stop=True)
                    sq = sm.tile([B, C], f32, tag="sq")
                    nc.vector.tensor_reduce(
                        sq[:], _view(spt[:], [(1, C), (C, 4)]), axis=AX.X,
                        op=op.add)
                    rt = sm.tile([B, C], f32, tag="rt")
                    nc.scalar.activation(rt[:], sq[:], ACT.Sqrt)
                    nc.vector.tensor_scalar(rt[:], rt[:], 1e-8, None,
                                            op0=op.add)
                    den = sm.tile([B, C], f32, tag="den")
                    nc.vector.tensor_scalar(den[:], sq[:], 1.0, None,
                                            op0=op.add)
                    nc.vector.tensor_tensor(den[:], den[:], rt[:], op=op.mult)
                    fi = sm.tile([B, C], f32, tag="fi")
                    nc.vector.reciprocal(fi[:], den[:])
                    nc.vector.tensor_tensor(fi[:], fi[:], sq[:], op=op.mult)
                    # transpose s back to [b, cm] and apply fi
                    if s_sbT.dtype != f16:
                        s16 = sm.tile([128, NCH * B], f16, tag="s16")
                        nc.vector.tensor_copy(s16[:], s_sbT[:])
                        s16v = s16
                    else:
                        s16v = s_sbT
                    vp = vps.tile([B, CM], f32, tag="vp")
                    for ci in range(NCH):
                        nc.tensor.matmul(
                            vp[:, ci * 128:(ci + 1) * 128],
                            s16v[:, ci * B:(ci + 1) * B], ident[:],
                            start=(ci % 4 == 0), stop=(ci % 4 == 3))
                    fi_b = _view(fi[:], [(0, M), (1, C)])
                    vp_v = _view(vp[:], [(C, M), (1, C)])
                    nc.vector.tensor_tensor(
                        _view(v_u[:], [(C, M), (1, C)]), vp_v, fi_b,
                        op=op.mult)
                    nc.vector.tensor_tensor(
                        _view(v_sb[:], [(C, M), (1, C)]), vp_v, fi_b,
                        op=op.mult)

            def allreduce_squash_T(sT_psum, scale):
                sT_sb = sm.tile([128, NCH * B], ar_dt, tag="sT_sb")
                nc.vector.tensor_copy(sT_sb[:], sT_psum[:])
                bin_ = drb.tile([128, NCH * B], ar_dt, tag="bin")
                bout = drb.tile([128, NCH * B], ar_dt, tag="bout")
                nc.sync.dma_start(bin_[:], sT_sb[:])
                if SIM_MODE:
                    nc.sync.dma_start(bout[:], bin_[:])
                else:
                    nc.gpsimd.collective_compute(
                        "AllReduce", op.add,
                        replica_groups=[list(range(N_CORES))],
                        ins=[bin_.opt()], outs=[bout.opt()],
                    )
                s2 = sm.tile([128, NCH * B], ar_dt, tag="s2")
                nc.sync.dma_start(s2[:], bout[:])
                squash_T(s2, scale)

            def emit_phase0(sT_pool):
                """s1 = sum_r u_r via one PSUM chain; transpose into s_T."""
                with (
                    tc.tile_pool(name="x0", bufs=3) as x0,
                    tc.tile_pool(name="w0", bufs=3) as w0,
                    tc.tile_pool(name="s1p", bufs=1, space="PSUM") as s1p,
                ):
                    s1_psum = s1p.tile([B, CM], f32)
                    for t in range(NT):
                        xt = x0.tile([2 * K, PAIRS * B], f16)
                        nc.sync.dma_start(
                            xt[:],
                            _ap(xT[t * PAIRS:(t + 1) * PAIRS],
                                [(B, 2 * K), (2 * K * B, PAIRS), (1, B)]))
                        wt = w0.tile([2 * K, PAIRS * CM], f16)
                        nc.sync.dma_start(
                            wt[:],
                            _ap(wT[t * PAIRS:(t + 1) * PAIRS],
                                [(CM, 2 * K), (2 * K * CM, PAIRS), (1, CM)]))
                        for j in range(PAIRS):
                            first = (t == 0 and j == 0)
                            last = (t == NT - 1 and j == PAIRS - 1)
                            for h in range(2):
                                nc.tensor.matmul(
                                    s1_psum[:, h * 512:(h + 1) * 512],
                                    xt[:, j * B:(j + 1) * B],
                                    wt[:, j * CM + h * 512:j * CM + (h + 1) * 512],
                                    start=first, stop=last,
                                )
                    s1_16 = sm.tile([B, CM], f16, tag="s1_16")
                    nc.scalar.copy(s1_16[:], s1_psum[:])
                    sT = sT_pool.tile([128, NCH * B], f32, tag="sT")
                    for ci in range(NCH):
                        nc.tensor.matmul(
                            sT[:, ci * B:(ci + 1) * B],
                            s1_16[:, ci * 128:(ci + 1) * 128], ident[:],
                            start=(ci % 4 == 0), stop=(ci % 4 == 3))
                return sT

            def dve_tile(t, ut, it, p_pool, q_pool, dp_pool, db_pool):
                """Vector-side routing for one tile; PE parts emitted
                separately (skewed). Returns (p, dotp, dotp32, dot_b, q)."""
                p = p_pool.tile([B, RT * CM], f16, tag="p")
                nc.vector.tensor_tensor(
                    p[:], ut[:], _view(v_u[:], [(0, RT), (1, CM)]), op=op.mult)
                dotp = dp_pool.tile([128, RT * B], f32)      # PSUM, per-r regions
                dotp32 = sm.tile([128, RT * B], f32, tag="dotp32")
                dot_b = db_pool.tile([B, RT * C], f32)       # PSUM
                return p, dotp, dotp32, dot_b

            def pe_dot(p, dotp, dotp32, dot_b):
                """sum_m on the PE: transpose-accumulate p chunks per route,
                drain via ACT, finish with the block-ones matmul."""
                for g in range(RT // 4):
                    for r in range(g * 4, g * 4 + 4):
                        for ci in range(NCH):
                            nc.tensor.matmul(
                                dotp[:, r * B:(r + 1) * B],
                                p[:, r * CM + ci * 128:r * CM + (ci + 1) * 128],
                                ident[:],
                                start=(r % 4 == 0 and ci == 0),
                                stop=(r % 4 == 3 and ci == NCH - 1))
                    nc.scalar.copy(dotp32[:, g * 4 * B:(g + 1) * 4 * B],
                                   dotp[:, g * 4 * B:(g + 1) * 4 * B])
                for r in range(RT):
                    nc.tensor.matmul(
                        dot_b[:, r * C:(r + 1) * C],
                        dotp32[:, r * B:(r + 1) * B], oblk[:],
                        start=(r == 0), stop=(r == RT - 1))

            def softmax_q(t, ut, it, dot_b, q_pool):
                """softmax over caps from dot_b (PSUM) + q-mult."""
                blt = b_log[:, t * RT * C:(t + 1) * RT * C]
                dot_v = _view(dot_b[:], [(C, RT), (1, C)])
                if it == 2:
                    lg_v = dot_v
                    if n_iters > 2:
                        nc.scalar.copy(_view(blt, [(C, RT), (1, C)]), dot_v)
                else:
                    lg = sm.tile([B, RT * C], f16, tag="lg")
                    lg_v = _view(lg[:], [(C, RT), (1, C)])
                    nc.vector.tensor_tensor(
                        lg_v, _view(blt, [(C, RT), (1, C)]), dot_v, op=op.add)
                    if it < n_iters:
                        nc.scalar.copy(_view(blt, [(C, RT), (1, C)]), lg_v)
                mx = sm.tile([B, RT], f32, tag="mx")
                nc.vector.tensor_reduce(mx[:], lg_v, axis=AX.X, op=op.max)
                e = sm.tile([B, RT * C], f32, tag="e")
                e_v = _view(e[:], [(C, RT), (1, C)])
                nc.vector.tensor_tensor(
                    e_v, lg_v, _view(mx[:], [(1, RT), (0, C)]), op=op.subtract)
                nc.scalar.activation(e[:], e[:], ACT.Exp)
                z = sm.tile([B, RT], f32, tag="z")
                nc.vector.tensor_reduce(z[:], e_v, axis=AX.X, op=op.add)
                nc.vector.reciprocal(z[:], z[:])
                coef = sm.tile([B, RT * C], f16, tag="coef")
                nc.vector.tensor_tensor(
                    _view(coef[:], [(C, RT), (1, C)]), e_v,
                    _view(z[:], [(1, RT), (0, C)]), op=op.mult)
                if DEBUG and it == n_iters:
                    nc.scalar.copy(_view(blt, [(C, RT), (1, C)]),
                                   _view(coef[:], [(C, RT), (1, C)]))
                q = q_pool.tile([B, RT * CM], f16, tag="q")
                # within-tile engine split: Pool takes routes [0, QPOOL_R),
                # DVE the rest -- keeps both engines evenly loaded per tile
                rp = QPOOL_R
                if rp > 0:
                    nc.gpsimd.tensor_tensor(
                        _view(q[:], [(CM, rp), (C, M), (1, C)]),
                        _view(ut[:], [(CM, rp), (C, M), (1, C)]),
                        _view(coef[:], [(C, rp), (0, M), (1, C)]),
                        op=op.mult)
                if rp < RT:
                    nc.vector.tensor_tensor(
                        _view(q[:, rp * CM:], [(CM, RT - rp), (C, M), (1, C)]),
                        _view(ut[:, rp * CM:], [(CM, RT - rp), (C, M), (1, C)]),
                        _view(coef[:, rp * C:], [(C, RT - rp), (0, M), (1, C)]),
                        op=op.mult)
                return q

            def pe_rsum(q, t, sT):
                """r-sum on the PE: transpose-accumulate q chunks into s_T."""
                for r in range(RT):
                    for ci in range(NCH):
                        # start/stop once per 2KB PSUM bank: start lazily
                        # marks the WHOLE bank pending-zero, so each bank
                        # must see exactly one start (its first write)
                        nc.tensor.matmul(
                            sT[:, ci * B:(ci + 1) * B],
                            q[:, r * CM + ci * 128:r * CM + (ci + 1) * 128],
                            ident[:],
                            start=(t == 0 and r == 0 and ci % 4 == 0),
                            stop=(t == NT - 1 and r == RT - 1 and ci % 4 == 3))

            def emit_phase1(sT):
                """u production fused with iteration-2 routing, software-
                pipelined: PE order is u-mms(t), dot-path(t-1), q-rsum(t-2)."""
                stage = {}
                with (
                    tc.tile_pool(name="x1", bufs=3) as x1,
                    tc.tile_pool(name="w1", bufs=2) as w1,
                    tc.tile_pool(name="up", bufs=3) as up,
                    tc.tile_pool(name="pp", bufs=3, space="PSUM") as pp,
                    tc.tile_pool(name="dpp", bufs=1, space="PSUM") as dpp,
                    tc.tile_pool(name="dbp", bufs=1, space="PSUM") as dbp,
                    tc.tile_pool(name="ppool", bufs=2) as p_pool,
                    tc.tile_pool(name="qpool", bufs=2) as q_pool,
                ):
                    for t in range(NT + 2):
                        if t < NT:
                            xt = x1.tile([2 * K, PAIRS * B], f16)
                            nc.sync.dma_start(
                                xt[:],
                                _ap(xT[t * PAIRS:(t + 1) * PAIRS],
                                    [(B, 2 * K), (2 * K * B, PAIRS), (1, B)]))
                            wt = w1.tile([2 * K, PAIRS * CM], f16)
                            nc.sync.dma_start(
                                wt[:],
                                _ap(wT[t * PAIRS:(t + 1) * PAIRS],
                                    [(CM, 2 * K), (2 * K * CM, PAIRS),
                                     (1, CM)]))
                            ut = up.tile([B, RT * CM], f16)
                            for r_idx in range(RT):
                                j, par = divmod(r_idx, 2)
                                for h in range(2):
                                    ps = pp.tile([B, 512], f32)
                                    nc.tensor.matmul(
                                        ps[:],
                                        xt[par * K:(par + 1) * K,
                                           j * B:(j + 1) * B],
                                        wt[par * K:(par + 1) * K,
                                           j * CM + h * 512:
                                           j * CM + (h + 1) * 512],
                                        start=True, stop=True,
                                    )
                                    nc.scalar.copy(
                                        ut[:, r_idx * CM + h * 512:
                                           r_idx * CM + (h + 1) * 512], ps[:])
                            nc.sync.dma_start(
                                u_dram[:, t * RT * CM:(t + 1) * RT * CM],
                                ut[:])
                            stage[t] = [ut, None, None]
                        if t - 1 >= 0 and t - 1 < NT:
                            ut1 = stage[t - 1][0]
                            p, dotp, dotp32, dot_b = dve_tile(
                                t - 1, ut1, 2, p_pool, q_pool, dpp, dbp)
                            pe_dot(p, dotp, dotp32, dot_b)
                            q = softmax_q(t - 1, ut1, 2, dot_b, q_pool)
                            stage[t - 1][1] = q
                        if t - 2 >= 0:
                            pe_rsum(stage[t - 2][1], t - 2, sT)
                            del stage[t - 2]

            def emit_phase2(it, sT):
                """One streaming routing pass over staged u (1-tile skew on
                the PE q-rsum)."""
                stage = {}
                with (
                    tc.tile_pool(name="up2", bufs=3) as up,
                    tc.tile_pool(name="dpp2", bufs=1, space="PSUM") as dpp,
                    tc.tile_pool(name="dbp2", bufs=1, space="PSUM") as dbp,
                    tc.tile_pool(name="ppool2", bufs=2) as p_pool,
                    tc.tile_pool(name="qpool2", bufs=2) as q_pool,
                ):
                    for t in range(NT + 1):
                        if t < NT:
                            ut = up.tile([B, RT * CM], f16)
                            nc.sync.dma_start(
                                ut[:],
                                u_dram[:, t * RT * CM:(t + 1) * RT * CM])
                            p, dotp, dotp32, dot_b = dve_tile(
                                t, ut, it, p_pool, q_pool, dpp, dbp)
                            pe_dot(p, dotp, dotp32, dot_b)
                            q = softmax_q(t, ut, it, dot_b, q_pool)
                            stage[t] = q
                        if t - 1 >= 0:
                            pe_rsum(stage[t - 1], t - 1, sT)
                            del stage[t - 1]

            def emit_once():
                with tc.tile_pool(name="sTp0", bufs=1, space="PSUM") as sTp:
                    sT = emit_phase0(sTp)
                    allreduce_squash_T(sT, 1.0 / C)
                if n_iters >= 2:
                    with tc.tile_pool(name="sTp1", bufs=1, space="PSUM") as sTp:
                        sT = sTp.tile([128, NCH * B], f32, tag="sT")
                        emit_phase1(sT)
                        allreduce_squash_T(sT, 1.0)
                for it in range(3, n_iters + 1):
                    with tc.tile_pool(name="sTp2", bufs=1, space="PSUM") as sTp:
                        sT = sTp.tile([128, NCH * B], f32, tag="sT")
                        emit_phase2(it, sT)
                        allreduce_squash_T(sT, 1.0)

            for _ in range(repeat):
                emit_once()

            nc.sync.dma_start(out[:], v_sb[:])
            if DEBUG:
                nc.sync.dma_start(dbg, b_log[:])

    nc.compile()
    return nc


def make_in_maps(x, w):
    """Host-side shard + layout prep: fp16, route pairs packed on 128
    partitions, weight columns (m, c) with c innermost."""
    ident = np.eye(128, dtype=np.float16)
    ident32 = np.eye(128, dtype=np.float32)
    oblk = np.zeros((128, C), dtype=np.float32)
    oblk[np.arange(128), np.arange(128) % C] = 1.0
    in_maps = []
    for c in range(N_CORES):
        sl = slice(c * R_LOC, (c + 1) * R_LOC)
        xT_c = np.ascontiguousarray(
            x[:, sl, :].transpose(1, 2, 0).reshape(R_LOC // 2, 2, K, B)
        ).astype(np.float16)
        wT_c = np.ascontiguousarray(
            w[sl].reshape(R_LOC // 2, 2, C, K, M).transpose(0, 1, 3, 4, 2)
        ).reshape(R_LOC // 2, 2, K, CM).astype(np.float16)
        in_maps.append({"xT": xT_c, "wT": wT_c, "idin": ident,
                        "idin32": ident32, "obin": oblk})
    return in_maps


def kernel(x, route_weights, num_iterations):
    global LAST_RESULT
    from concourse import bass_utils

    n = int(num_iterations)
    assert n >= 1
    x = np.asarray(x, dtype=np.float32)
    w = np.asarray(route_weights, dtype=np.float32)
    assert x.shape == (B, R, K) and w.shape == (R, C, K, M)

    if n not in _compiled:
        _compiled[n] = _build(n)
    nc = _compiled[n]

    in_maps = make_in_maps(x, w)
    res = bass_utils.run_bass_kernel_spmd(
        nc, in_maps, core_ids=list(range(N_CORES)))
    LAST_RESULT = res
    return np.ascontiguousarray(
        res.results[0]["out"].reshape(B, M, C).transpose(0, 2, 1)
    ).astype(np.float32)
